# revision 64
# baseline (speedup 1.0000x reference)
"""Causal self-attention (per-head A projections) on 8 TRN2 NeuronCores.

Shapes: h [B=2, T=2048, d=64] f32, A [H=8, d, d] f32.
  q = h @ A[i]; scores = q @ h^T (causal); out_i = softmax(scores) @ h.
Sharding: one head per core (embarrassingly parallel, no collectives).
Each core receives the full h and its own A[i]; host concatenates heads.

Kernel structure per core (phases run for BOTH batches in sequence so the
matmul-dense pass-2/AV stretch is one long run that keeps the PE clock warm):
  Phase A: chunked DMA h -> SBUF; TensorE-transpose into per-chunk hT tiles
           [65, 512] f32r (64 data rows + ones row); qT = A-matmul, stored
           f32r (rounded for the single-pass fp32 matmul mode) + bf16 copy.
  Stats:   scores[t,s] bf16 tiles (lhsT=qT16 tile, rhs=hT16 chunk); causal
           diag handled by ACCUMULATING a -1e30 upper-triangular matrix via
           an extra identity-matmul into the same PSUM (no elementwise mask);
           DVE reduce_max(negate) -> -m; a tiny SBUF->SBUF DMA reshapes the
           [128,1] column into the 65th row of the qT chunk tile.
  Pass 2:  scoresT[s,t] f32r tiles with K=65 (the -m row makes the matmul
           compute scores - m directly); ACT exp reads PSUM and writes pT
           bf16 straight to SBUF (no transposes of p anywhere); gpsimd
           affine_select zeroes the acausal part of diagonal tiles.
  AV:      outT[c] [65, t] = sum_j [h_j | 1]^T @ pT(c,j) accumulated in PSUM
           (lhsT = hs16_j is stationary per j -> few LDWEIGHTS); the ones
           column of hs16 makes row 64 the softmax denominator l.  AV for
           tile (c,j) is emitted two tiles late (software pipeline) so the
           in-order PE always has independent score matmuls in its queue
           while ACT computes this tile's exp.
  Final:   outT chunk -> SBUF, small TensorE transposes [65,128]->[128,65]
           carry l along; DVE reciprocal + tensor_scalar_mul; DMA out.
"""

import sys

for _p in ("/opt/trn_rl_repo",):
    if _p not in sys.path:
        sys.path.insert(0, _p)

import numpy as np
from contextlib import ExitStack

import concourse.bass as bass
import concourse.tile as tile
from concourse import bacc, mybir
from concourse.masks import make_identity
from concourse.bass_utils import run_bass_kernel_spmd

B, T, D, H = 2, 2048, 64, 8
P = 128                # square tile size (t and s)
NT = T // P            # 16 tiles along t/s
CH = 512               # chunk width (PSUM bank / fp32 moving max)
NCH = T // CH          # 4 chunks
NEG = -1e30

f32 = mybir.dt.float32
f32r = mybir.dt.float32r
bf16 = mybir.dt.bfloat16

# Pass-2 score matmul mode: "f32" (4 cyc/row, exact), "f32r" (1 cyc/row,
# reduced precision single-pass).  Stats pass is always bf16 (only feeds the
# row-max bound, where +-1 error is harmless).
PASS2_MODE = "f32r"


def _ceil_div(a, b):
    return (a + b - 1) // b


def _build(ctx: ExitStack, tc: "tile.TileContext", h_ext, A_ext, out_ext):
    nc = tc.nc

    consts = ctx.enter_context(tc.tile_pool(name="consts", bufs=1))
    hpool = ctx.enter_context(tc.tile_pool(name="hpool", bufs=2))
    qpool = ctx.enter_context(tc.tile_pool(name="qpool", bufs=2))
    ppool = ctx.enter_context(tc.tile_pool(name="ppool", bufs=8))
    spool = ctx.enter_context(tc.tile_pool(name="spool", bufs=4))
    opool = ctx.enter_context(tc.tile_pool(name="opool", bufs=3))
    # PSUM budget (8 banks): sc 4 (stats chunks + pass-2 tiles) + oT 2x1 + misc 2.
    ps_sc = ctx.enter_context(tc.tile_pool(name="ps_sc", bufs=4, space="PSUM"))
    ps_oT = ctx.enter_context(tc.tile_pool(name="ps_oT", bufs=1, space="PSUM"))
    ps_misc = ctx.enter_context(tc.tile_pool(name="ps_misc", bufs=2, space="PSUM"))

    # ---- constants ----
    ident = consts.tile([P, P], f32)
    make_identity(nc, ident)
    identb = consts.tile([P, P], bf16)
    make_identity(nc, identb)

    # Umask[t, s] = NEG where s > t else 0 (stats-pass diagonal tile).
    umask = consts.tile([P, P], bf16)
    nc.gpsimd.memset(umask, 0.0)
    nc.gpsimd.affine_select(
        out=umask, in_=umask, compare_op=mybir.AluOpType.is_ge, fill=NEG,
        base=0, channel_multiplier=1, pattern=[[-1, P]],
    )

    # A for this core's head: [d, e] natural layout (d on partitions).
    p2dt = f32r if PASS2_MODE == "f32r" else f32
    Asb32 = consts.tile([D, D], f32)
    nc.sync.dma_start(out=Asb32, in_=A_ext)
    if PASS2_MODE == "f32r":
        Asb = consts.tile([D, D], f32r)
        nc.scalar.copy(Asb, Asb32)
    else:
        Asb = Asb32

    # Prefetch both batches' h up front (sync queue carries only bulk loads).
    hs32s = []
    hs16s = []
    for b in range(B):
        hs32 = hpool.tile([P, NT, D + 1], f32, tag="hs32", name=f"hs32_{b}")
        h_re = h_ext[b].rearrange("(j p) d -> p j d", p=P)
        for c in range(NCH):
            nc.sync.dma_start(
                out=hs32[:, 4 * c : 4 * c + 4, 0:D], in_=h_re[:, 4 * c : 4 * c + 4, :]
            )
        nc.gpsimd.memset(hs32[:, :, D : D + 1], 1.0)
        hs16 = hpool.tile([P, NT, D + 1], bf16, tag="hs16", name=f"hs16_{b}")
        nc.vector.tensor_copy(hs16, hs32)
        hs32s.append(hs32)
        hs16s.append(hs16)

    all_hTs = []
    all_qTs = []
    for b in range(B):
        hs32 = hs32s[b]
        # hT / qT built per 512-wide chunk so stats can start before the whole
        # transpose phase finishes.
        hTs = []
        hT16s = []
        qTs = []
        qT16s = []
        for c in range(NCH):
            hc = hpool.tile([D + 1, CH], p2dt, tag=f"hT{c}", name=f"hT_{b}_{c}")
            for r in range(4):
                j = 4 * c + r
                pt = ps_misc.tile([D + 1, P], f32, tag="misc")
                nc.tensor.transpose(pt, hs32[:, j, :], ident)
                nc.scalar.copy(hc[:, r * P : (r + 1) * P], pt)
            hc16 = hpool.tile([D, CH], bf16, tag=f"hT16_{c}", name=f"hT16_{b}_{c}")
            nc.vector.tensor_copy(hc16, hc[0:D, :])
            hTs.append(hc)
            hT16s.append(hc16)

            # qT chunk [65, CH]: rows 0..63 = qT, row 64 = -m (written later).
            qc = qpool.tile([D + 1, CH], p2dt, tag=f"qT{c}", name=f"qT_{b}_{c}")
            pq = ps_misc.tile([D, CH], f32, tag="misc")
            nc.tensor.matmul(
                pq, lhsT=Asb, rhs=hc[0:D, :], start=True, stop=True,
            )
            nc.scalar.copy(qc[0:D, :], pq)
            qc16 = qpool.tile([D, CH], bf16, tag=f"qT16_{c}", name=f"qT16_{b}_{c}")
            nc.scalar.copy(qc16, pq)
            qTs.append(qc)
            qT16s.append(qc16)

        # ---- Stats pass: row max per t-tile ----
        for i in range(NT):
            s_end = (i + 1) * P
            nchunks = _ceil_div(s_end, CH)
            mxp = spool.tile([P, 4], f32, tag="mxp")
            lhs_q = qT16s[i // 4][:, (i % 4) * P : (i % 4 + 1) * P]
            for c in range(nchunks):
                w = min(CH, s_end - c * CH)
                ps = ps_sc.tile([P, CH], f32, tag="sc")
                diag = c == nchunks - 1
                nc.tensor.matmul(
                    ps[:, 0:w], lhsT=lhs_q, rhs=hT16s[c][:, 0:w],
                    start=True, stop=not diag, skip_group_check=True,
                )
                if diag:
                    nc.tensor.matmul(
                        ps[:, w - P : w], lhsT=identb, rhs=umask,
                        start=False, stop=True, skip_group_check=True,
                    )
                nc.vector.reduce_max(
                    mxp[:, c : c + 1], ps[:, 0:w], axis=mybir.AxisListType.X
                )
            negm = spool.tile([P, 1], p2dt, tag="negm")
            if nchunks > 1:
                nc.vector.reduce_max(
                    negm, mxp[:, 0:nchunks], axis=mybir.AxisListType.X, negate=True
                )
            else:
                nc.vector.tensor_scalar_mul(negm, mxp[:, 0:1], -1.0)
            # Partition-column -> free-row reshape via a tiny SBUF->SBUF DMA
            # (keeping this off TensorE avoids transpose-mode matmuls).
            nc.sync.dma_start(
                out=qTs[i // 4][D : D + 1, (i % 4) * P : (i % 4 + 1) * P],
                in_=negm,
            )

        all_hTs.append(hTs)
        all_qTs.append(qTs)

    # ---- Pass 2 + AV for both batches: one long matmul-dense stretch ----
    # outT[c] accumulates [d'+1, t] = sum_j [h_j | 1]^T @ pT(c,j); the
    # ones column of hs16 makes row 64 the softmax denominator l.
    for b in range(B):
        hs16 = hs16s[b]
        hTs = all_hTs[b]
        qTs = all_qTs[b]
        for grp in reversed(range(NCH // 2)):
            cs = [2 * grp, 2 * grp + 1]
            oTs = {
                c: ps_oT.tile(
                    [D + 1, CH], f32, tag=f"oT{c % 2}", name=f"oT_g{grp}_c{c}"
                )
                for c in cs
            }
            # Software-pipeline: emit each tile's AV two tiles late so the PE
            # (in-order) has independent score matmuls to chew on while ACT
            # computes the exp for this tile.
            av_queue = []

            def finalize_chunk(cq):
                # Runs as soon as chunk cq's accumulation stops, overlapping
                # the remaining AV/score matmuls of the group.
                oc = opool.tile([D + 1, CH], f32, tag="oc", name=f"oc_{b}_{cq}")
                nc.vector.tensor_copy(oc, oTs[cq])
                for r in range(4):
                    ii = 4 * cq + r
                    ot = ps_misc.tile([P, D + 1], f32, tag="misc")
                    nc.tensor.transpose(
                        ot, oc[:, r * P : (r + 1) * P], ident[0 : D + 1, 0 : D + 1]
                    )
                    rl = spool.tile([P, 1], f32, tag="rl")
                    nc.vector.reciprocal(rl, ot[:, D : D + 1])
                    osb = opool.tile([P, D], f32, tag="osb")
                    nc.vector.tensor_scalar_mul(osb, ot[:, 0:D], rl)
                    nc.sync.dma_start(
                        out=out_ext[b, ii * P : (ii + 1) * P, :], in_=osb
                    )

            def flush_av(limit):
                while len(av_queue) > limit:
                    cq, jq, pTq = av_queue.pop(0)
                    nc.tensor.matmul(
                        oTs[cq], lhsT=hs16[:, jq, :], rhs=pTq,
                        start=(jq == 0), stop=(jq == 4 * cq + 3),
                        skip_group_check=True,
                    )
                    if jq == 4 * cq + 3:
                        finalize_chunk(cq)

            for j in range(4 * cs[-1] + 4):
                for c in cs:
                    if j > 4 * c + 3:
                        continue
                    p2 = ps_sc.tile([P, CH], f32, tag="sc")
                    nc.tensor.matmul(
                        p2,
                        lhsT=hTs[j // 4][:, (j % 4) * P : (j % 4 + 1) * P],
                        rhs=qTs[c][:, :],
                        start=True, stop=True, skip_group_check=True,
                    )
                    pT = ppool.tile([P, CH], bf16, tag="pT")
                    nc.scalar.activation(pT, p2, mybir.ActivationFunctionType.Exp)
                    if j >= 4 * c:
                        # zero the acausal part (s_global > t_global); the AV
                        # pipeline delay gives gpsimd slack to do this.
                        nc.gpsimd.affine_select(
                            out=pT, in_=pT, compare_op=mybir.AluOpType.is_ge,
                            fill=0.0, base=-P * (j - 4 * c),
                            channel_multiplier=-1, pattern=[[1, CH]],
                        )
                    av_queue.append((c, j, pT))
                    flush_av(2)
            flush_av(0)


_cache = {}


def _get_nc():
    if "nc" not in _cache:
        nc = bacc.Bacc(
            "TRN2", target_bir_lowering=False, debug=False, num_devices=H
        )
        h_ext = nc.dram_tensor("h", [B, T, D], f32, kind="ExternalInput").ap()
        A_ext = nc.dram_tensor("A", [D, D], f32, kind="ExternalInput").ap()
        out_ext = nc.dram_tensor("out", [B, T, D], f32, kind="ExternalOutput").ap()
        with tile.TileContext(nc) as tc:
            with ExitStack() as ctx:
                _build(ctx, tc, h_ext, A_ext, out_ext)
        nc.compile()
        _cache["nc"] = nc
    return _cache["nc"]


def run(h, A, **kw):
    """Run on hardware; returns (full output [B,T,H*D], BassKernelResults)."""
    nc = _get_nc()
    h = np.ascontiguousarray(h, dtype=np.float32)
    A = np.ascontiguousarray(A, dtype=np.float32)
    in_maps = [{"h": h, "A": np.ascontiguousarray(A[i])} for i in range(H)]
    res = run_bass_kernel_spmd(nc, in_maps, core_ids=list(range(H)), **kw)
    out = np.concatenate([res.results[i]["out"] for i in range(H)], axis=-1)
    return out, res


def kernel(h, A):
    out, _ = run(h, A)
    return out


# revision 65
# speedup vs baseline: 1.1621x; 1.1621x over previous
"""Causal self-attention (per-head A projections) on 8 TRN2 NeuronCores.

Shapes: h [B=2, T=2048, d=64] f32, A [H=8, d, d] f32.
  q = h @ A[i]; scores = q @ h^T (causal); out_i = softmax(scores) @ h.
Sharding: one head per core (embarrassingly parallel, no collectives).
Each core receives the full h and its own A[i]; host concatenates heads.

Kernel structure per core (phases run for BOTH batches in sequence so the
matmul-dense pass-2/AV stretch is one long run that keeps the PE clock warm):
  Phase A: chunked DMA h -> SBUF; TensorE-transpose into per-chunk hT tiles
           [65, 512] f32r (64 data rows + ones row); qT = A-matmul, stored
           f32r (rounded for the single-pass fp32 matmul mode) + bf16 copy.
  Stats:   scores[t,s] bf16 tiles (lhsT=qT16 tile, rhs=hT16 chunk); causal
           diag handled by ACCUMULATING a -1e30 upper-triangular matrix via
           an extra identity-matmul into the same PSUM (no elementwise mask);
           DVE reduce_max(negate) -> -m; a tiny SBUF->SBUF DMA reshapes the
           [128,1] column into the 65th row of the qT chunk tile.
  Pass 2:  scoresT[s,t] f32r tiles with K=65 (the -m row makes the matmul
           compute scores - m directly); ACT exp reads PSUM and writes pT
           bf16 straight to SBUF (no transposes of p anywhere); gpsimd
           affine_select zeroes the acausal part of diagonal tiles.
  AV:      outT[c] [65, t] = sum_j [h_j | 1]^T @ pT(c,j) accumulated in PSUM
           (lhsT = hs16_j is stationary per j -> few LDWEIGHTS); the ones
           column of hs16 makes row 64 the softmax denominator l.  AV for
           tile (c,j) is emitted two tiles late (software pipeline) so the
           in-order PE always has independent score matmuls in its queue
           while ACT computes this tile's exp.
  Final:   outT chunk -> SBUF, small TensorE transposes [65,128]->[128,65]
           carry l along; DVE reciprocal + tensor_scalar_mul; DMA out.
"""

import sys

for _p in ("/opt/trn_rl_repo",):
    if _p not in sys.path:
        sys.path.insert(0, _p)

import numpy as np
from contextlib import ExitStack

import concourse.bass as bass
import concourse.tile as tile
from concourse import bacc, mybir
from concourse.masks import make_identity
from concourse.bass_utils import run_bass_kernel_spmd

B, T, D, H = 2, 2048, 64, 8
P = 128                # square tile size (t and s)
NT = T // P            # 16 tiles along t/s
CH = 512               # chunk width (PSUM bank / fp32 moving max)
NCH = T // CH          # 4 chunks
NEG = -1e30

f32 = mybir.dt.float32
f32r = mybir.dt.float32r
bf16 = mybir.dt.bfloat16

# Pass-2 score matmul mode: "f32" (4 cyc/row, exact), "f32r" (1 cyc/row,
# reduced precision single-pass).  Stats pass is always bf16 (only feeds the
# row-max bound, where +-1 error is harmless).
PASS2_MODE = "f32r"


def _ceil_div(a, b):
    return (a + b - 1) // b


def _build(ctx: ExitStack, tc: "tile.TileContext", h_ext, A_ext, out_ext):
    nc = tc.nc

    consts = ctx.enter_context(tc.tile_pool(name="consts", bufs=1))
    hpool = ctx.enter_context(tc.tile_pool(name="hpool", bufs=2))
    qpool = ctx.enter_context(tc.tile_pool(name="qpool", bufs=2))
    ppool = ctx.enter_context(tc.tile_pool(name="ppool", bufs=8))
    spool = ctx.enter_context(tc.tile_pool(name="spool", bufs=4))
    opool = ctx.enter_context(tc.tile_pool(name="opool", bufs=3))
    # PSUM budget (8 banks): sc 4 (stats chunks + pass-2 tiles) + oT 2x1 + misc 2.
    ps_sc = ctx.enter_context(tc.tile_pool(name="ps_sc", bufs=4, space="PSUM"))
    ps_oT = ctx.enter_context(tc.tile_pool(name="ps_oT", bufs=1, space="PSUM"))
    ps_misc = ctx.enter_context(tc.tile_pool(name="ps_misc", bufs=2, space="PSUM"))

    # ---- constants ----
    ident = consts.tile([P, P], f32)
    make_identity(nc, ident)
    identb = consts.tile([P, P], bf16)
    make_identity(nc, identb)

    # Umask[t, s] = NEG where s > t else 0 (stats-pass diagonal tile).
    umask = consts.tile([P, P], bf16)
    nc.gpsimd.memset(umask, 0.0)
    nc.gpsimd.affine_select(
        out=umask, in_=umask, compare_op=mybir.AluOpType.is_ge, fill=NEG,
        base=0, channel_multiplier=1, pattern=[[-1, P]],
    )

    # A for this core's head: [d, e] natural layout (d on partitions).
    p2dt = f32r if PASS2_MODE == "f32r" else f32
    Asb32 = consts.tile([D, D], f32)
    nc.sync.dma_start(out=Asb32, in_=A_ext)
    if PASS2_MODE == "f32r":
        Asb = consts.tile([D, D], f32r)
        nc.scalar.copy(Asb, Asb32)
    else:
        Asb = Asb32

    # Prefetch both batches' h up front (sync queue carries only bulk loads).
    hs32s = []
    hs16s = []
    for b in range(B):
        hs32 = hpool.tile([P, NT, D + 1], f32, tag="hs32", name=f"hs32_{b}")
        h_re = h_ext[b].rearrange("(j p) d -> p j d", p=P)
        for c in range(NCH):
            nc.sync.dma_start(
                out=hs32[:, 4 * c : 4 * c + 4, 0:D], in_=h_re[:, 4 * c : 4 * c + 4, :]
            )
        nc.gpsimd.memset(hs32[:, :, D : D + 1], 1.0)
        hs16 = hpool.tile([P, NT, D + 1], bf16, tag="hs16", name=f"hs16_{b}")
        nc.vector.tensor_copy(hs16, hs32)
        hs32s.append(hs32)
        hs16s.append(hs16)

    all_hTs = []
    all_qTs = []
    for b in range(B):
        hs32 = hs32s[b]
        # hT / qT built per 512-wide chunk so stats can start before the whole
        # transpose phase finishes.
        hTs = []
        hT16s = []
        qTs = []
        qT16s = []
        for c in range(NCH):
            hc = hpool.tile([D + 1, CH], p2dt, tag=f"hT{c}", name=f"hT_{b}_{c}")
            for r in range(4):
                j = 4 * c + r
                pt = ps_misc.tile([D + 1, P], f32, tag="misc")
                nc.tensor.transpose(pt, hs32[:, j, :], ident)
                nc.scalar.copy(hc[:, r * P : (r + 1) * P], pt)
            hc16 = hpool.tile([D, CH], bf16, tag=f"hT16_{c}", name=f"hT16_{b}_{c}")
            nc.vector.tensor_copy(hc16, hc[0:D, :])
            hTs.append(hc)
            hT16s.append(hc16)

            # qT chunk [65, CH]: rows 0..63 = qT, row 64 = -m (written later).
            qc = qpool.tile([D + 1, CH], p2dt, tag=f"qT{c}", name=f"qT_{b}_{c}")
            pq = ps_misc.tile([D, CH], f32, tag="misc")
            nc.tensor.matmul(
                pq, lhsT=Asb, rhs=hc[0:D, :], start=True, stop=True,
            )
            nc.scalar.copy(qc[0:D, :], pq)
            qc16 = qpool.tile([D, CH], bf16, tag=f"qT16_{c}", name=f"qT16_{b}_{c}")
            nc.scalar.copy(qc16, pq)
            qTs.append(qc)
            qT16s.append(qc16)

        # ---- Stats pass: row max per t-tile ----
        for i in range(NT):
            s_end = (i + 1) * P
            nchunks = _ceil_div(s_end, CH)
            mxp = spool.tile([P, 4], f32, tag="mxp")
            lhs_q = qT16s[i // 4][:, (i % 4) * P : (i % 4 + 1) * P]
            for c in range(nchunks):
                w = min(CH, s_end - c * CH)
                ps = ps_sc.tile([P, CH], f32, tag="sc")
                diag = c == nchunks - 1
                nc.tensor.matmul(
                    ps[:, 0:w], lhsT=lhs_q, rhs=hT16s[c][:, 0:w],
                    start=True, stop=not diag, skip_group_check=True,
                )
                if diag:
                    nc.tensor.matmul(
                        ps[:, w - P : w], lhsT=identb, rhs=umask,
                        start=False, stop=True, skip_group_check=True,
                    )
                nc.vector.reduce_max(
                    mxp[:, c : c + 1], ps[:, 0:w], axis=mybir.AxisListType.X
                )
            negm = spool.tile([P, 1], p2dt, tag="negm")
            if nchunks > 1:
                nc.vector.reduce_max(
                    negm, mxp[:, 0:nchunks], axis=mybir.AxisListType.X, negate=True
                )
            else:
                nc.vector.tensor_scalar_mul(negm, mxp[:, 0:1], -1.0)
            # Partition-column -> free-row reshape via a tiny SBUF->SBUF DMA
            # (keeping this off TensorE avoids transpose-mode matmuls).
            nc.sync.dma_start(
                out=qTs[i // 4][D : D + 1, (i % 4) * P : (i % 4 + 1) * P],
                in_=negm,
            )

        all_hTs.append(hTs)
        all_qTs.append(qTs)

    # ---- Pass 2 + AV for both batches: one long matmul-dense stretch ----
    # outT[c] accumulates [d'+1, t] = sum_j [h_j | 1]^T @ pT(c,j); the
    # ones column of hs16 makes row 64 the softmax denominator l.
    for b in range(B):
        hs16 = hs16s[b]
        hTs = all_hTs[b]
        qTs = all_qTs[b]
        for grp in range(NCH // 2):
            cs = [2 * grp, 2 * grp + 1]
            oTs = {
                c: ps_oT.tile(
                    [D + 1, CH], f32, tag=f"oT{c % 2}", name=f"oT_g{grp}_c{c}"
                )
                for c in cs
            }
            # Software-pipeline: emit each tile's AV two tiles late so the PE
            # (in-order) has independent score matmuls to chew on while ACT
            # computes the exp for this tile.
            av_queue = []

            def finalize_chunk(cq):
                # Runs as soon as chunk cq's accumulation stops, overlapping
                # the remaining AV/score matmuls of the group.
                oc = opool.tile([D + 1, CH], f32, tag="oc", name=f"oc_{b}_{cq}")
                nc.vector.tensor_copy(oc, oTs[cq])
                for r in range(4):
                    ii = 4 * cq + r
                    ot = ps_misc.tile([P, D + 1], f32, tag="misc")
                    nc.tensor.transpose(
                        ot, oc[:, r * P : (r + 1) * P], ident[0 : D + 1, 0 : D + 1]
                    )
                    rl = spool.tile([P, 1], f32, tag="rl")
                    nc.vector.reciprocal(rl, ot[:, D : D + 1])
                    osb = opool.tile([P, D], f32, tag="osb")
                    nc.vector.tensor_scalar_mul(osb, ot[:, 0:D], rl)
                    nc.sync.dma_start(
                        out=out_ext[b, ii * P : (ii + 1) * P, :], in_=osb
                    )

            def flush_av(limit):
                while len(av_queue) > limit:
                    cq, jq, pTq = av_queue.pop(0)
                    nc.tensor.matmul(
                        oTs[cq], lhsT=hs16[:, jq, :], rhs=pTq,
                        start=(jq == 0), stop=(jq == 4 * cq + 3),
                        skip_group_check=True,
                    )
                    if jq == 4 * cq + 3:
                        finalize_chunk(cq)

            for j in range(4 * cs[-1] + 4):
                for c in cs:
                    if j > 4 * c + 3:
                        continue
                    p2 = ps_sc.tile([P, CH], f32, tag="sc")
                    nc.tensor.matmul(
                        p2,
                        lhsT=hTs[j // 4][:, (j % 4) * P : (j % 4 + 1) * P],
                        rhs=qTs[c][:, :],
                        start=True, stop=True, skip_group_check=True,
                    )
                    pT = ppool.tile([P, CH], bf16, tag="pT")
                    nc.scalar.activation(pT, p2, mybir.ActivationFunctionType.Exp)
                    if j >= 4 * c:
                        # zero the acausal part (s_global > t_global); the AV
                        # pipeline delay gives gpsimd slack to do this.
                        nc.gpsimd.affine_select(
                            out=pT, in_=pT, compare_op=mybir.AluOpType.is_ge,
                            fill=0.0, base=-P * (j - 4 * c),
                            channel_multiplier=-1, pattern=[[1, CH]],
                        )
                    av_queue.append((c, j, pT))
                    flush_av(2)
            flush_av(0)


_cache = {}


def _get_nc():
    if "nc" not in _cache:
        nc = bacc.Bacc(
            "TRN2", target_bir_lowering=False, debug=False, num_devices=H
        )
        h_ext = nc.dram_tensor("h", [B, T, D], f32, kind="ExternalInput").ap()
        A_ext = nc.dram_tensor("A", [D, D], f32, kind="ExternalInput").ap()
        out_ext = nc.dram_tensor("out", [B, T, D], f32, kind="ExternalOutput").ap()
        with tile.TileContext(nc) as tc:
            with ExitStack() as ctx:
                _build(ctx, tc, h_ext, A_ext, out_ext)
        nc.compile()
        _cache["nc"] = nc
    return _cache["nc"]


def run(h, A, **kw):
    """Run on hardware; returns (full output [B,T,H*D], BassKernelResults)."""
    nc = _get_nc()
    h = np.ascontiguousarray(h, dtype=np.float32)
    A = np.ascontiguousarray(A, dtype=np.float32)
    in_maps = [{"h": h, "A": np.ascontiguousarray(A[i])} for i in range(H)]
    res = run_bass_kernel_spmd(nc, in_maps, core_ids=list(range(H)), **kw)
    out = np.concatenate([res.results[i]["out"] for i in range(H)], axis=-1)
    return out, res


def kernel(h, A):
    out, _ = run(h, A)
    return out


# revision 66
# speedup vs baseline: 1.1862x; 1.0207x over previous
"""Causal self-attention (per-head A projections) on 8 TRN2 NeuronCores.

Shapes: h [B=2, T=2048, d=64] f32, A [H=8, d, d] f32.
  q = h @ A[i]; scores = q @ h^T (causal); out_i = softmax(scores) @ h.
Sharding: one head per core (embarrassingly parallel, no collectives).
Each core receives the full h and its own A[i]; host concatenates heads.

Kernel structure per core (phases run for BOTH batches in sequence so the
matmul-dense pass-2/AV stretch is one long run that keeps the PE clock warm):
  Phase A: chunked DMA h -> SBUF; TensorE-transpose into per-chunk hT tiles
           [65, 512] f32r (64 data rows + ones row); qT = A-matmul, stored
           f32r (rounded for the single-pass fp32 matmul mode) + bf16 copy.
  Stats:   scores[t,s] bf16 tiles (lhsT=qT16 tile, rhs=hT16 chunk); causal
           diag handled by ACCUMULATING a -1e30 upper-triangular matrix via
           an extra identity-matmul into the same PSUM (no elementwise mask);
           DVE reduce_max(negate) -> -m; a tiny SBUF->SBUF DMA reshapes the
           [128,1] column into the 65th row of the qT chunk tile.
  Pass 2:  scoresT[s,t] f32r tiles with K=65 (the -m row makes the matmul
           compute scores - m directly); ACT exp reads PSUM and writes pT
           bf16 straight to SBUF (no transposes of p anywhere); gpsimd
           affine_select zeroes the acausal part of diagonal tiles.
  AV:      outT[c] [65, t] = sum_j [h_j | 1]^T @ pT(c,j) accumulated in PSUM
           (lhsT = hs16_j is stationary per j -> few LDWEIGHTS); the ones
           column of hs16 makes row 64 the softmax denominator l.  AV for
           tile (c,j) is emitted two tiles late (software pipeline) so the
           in-order PE always has independent score matmuls in its queue
           while ACT computes this tile's exp.
  Final:   outT chunk -> SBUF, small TensorE transposes [65,128]->[128,65]
           carry l along; DVE reciprocal + tensor_scalar_mul; DMA out.
"""

import sys

for _p in ("/opt/trn_rl_repo",):
    if _p not in sys.path:
        sys.path.insert(0, _p)

import numpy as np
from contextlib import ExitStack

import concourse.bass as bass
import concourse.tile as tile
from concourse import bacc, mybir
from concourse.masks import make_identity
from concourse.bass_utils import run_bass_kernel_spmd

B, T, D, H = 2, 2048, 64, 8
P = 128                # square tile size (t and s)
NT = T // P            # 16 tiles along t/s
CH = 512               # chunk width (PSUM bank / fp32 moving max)
NCH = T // CH          # 4 chunks
NEG = -1e30

f32 = mybir.dt.float32
f32r = mybir.dt.float32r
bf16 = mybir.dt.bfloat16

# Pass-2 score matmul mode: "f32" (4 cyc/row, exact), "f32r" (1 cyc/row,
# reduced precision single-pass).  Stats pass is always bf16 (only feeds the
# row-max bound, where +-1 error is harmless).
PASS2_MODE = "f32r"


def _ceil_div(a, b):
    return (a + b - 1) // b


def _build(ctx: ExitStack, tc: "tile.TileContext", h_ext, A_ext, out_ext):
    nc = tc.nc

    consts = ctx.enter_context(tc.tile_pool(name="consts", bufs=1))
    hpool = ctx.enter_context(tc.tile_pool(name="hpool", bufs=2))
    qpool = ctx.enter_context(tc.tile_pool(name="qpool", bufs=2))
    ppool = ctx.enter_context(tc.tile_pool(name="ppool", bufs=8))
    spool = ctx.enter_context(tc.tile_pool(name="spool", bufs=4))
    opool = ctx.enter_context(tc.tile_pool(name="opool", bufs=3))
    # PSUM budget (8 banks): sc 4 (stats chunks + pass-2 tiles) + oT 2x1 + misc 2.
    ps_sc = ctx.enter_context(tc.tile_pool(name="ps_sc", bufs=4, space="PSUM"))
    ps_oT = ctx.enter_context(tc.tile_pool(name="ps_oT", bufs=1, space="PSUM"))
    ps_misc = ctx.enter_context(tc.tile_pool(name="ps_misc", bufs=2, space="PSUM"))

    # ---- constants ----
    ident = consts.tile([P, P], f32)
    make_identity(nc, ident)
    identb = consts.tile([P, P], bf16)
    make_identity(nc, identb)

    # Umask[t, s] = NEG where s > t else 0 (stats-pass diagonal tile).
    umask = consts.tile([P, P], bf16)
    nc.gpsimd.memset(umask, 0.0)
    nc.gpsimd.affine_select(
        out=umask, in_=umask, compare_op=mybir.AluOpType.is_ge, fill=NEG,
        base=0, channel_multiplier=1, pattern=[[-1, P]],
    )

    # A for this core's head: [d, e] natural layout (d on partitions).
    p2dt = f32r if PASS2_MODE == "f32r" else f32
    Asb32 = consts.tile([D, D], f32)
    nc.sync.dma_start(out=Asb32, in_=A_ext)
    if PASS2_MODE == "f32r":
        Asb = consts.tile([D, D], f32r)
        nc.scalar.copy(Asb, Asb32)
    else:
        Asb = Asb32

    # Prefetch both batches' h up front (sync queue carries only bulk loads).
    hs32s = []
    hs16s = []
    for b in range(B):
        hs32 = hpool.tile([P, NT, D + 1], f32, tag="hs32", name=f"hs32_{b}")
        h_re = h_ext[b].rearrange("(j p) d -> p j d", p=P)
        for c in range(NCH):
            nc.sync.dma_start(
                out=hs32[:, 4 * c : 4 * c + 4, 0:D], in_=h_re[:, 4 * c : 4 * c + 4, :]
            )
        nc.gpsimd.memset(hs32[:, :, D : D + 1], 1.0)
        hs16 = hpool.tile([P, NT, D + 1], bf16, tag="hs16", name=f"hs16_{b}")
        nc.vector.tensor_copy(hs16, hs32)
        hs32s.append(hs32)
        hs16s.append(hs16)

    all_hTs = []
    all_qTs = []
    for b in range(B):
        hs32 = hs32s[b]
        # hT / qT built per 512-wide chunk so stats can start before the whole
        # transpose phase finishes.
        hTs = []
        hT16s = []
        qTs = []
        qT16s = []
        for c in range(NCH):
            hc = hpool.tile([D + 1, CH], p2dt, tag=f"hT{c}", name=f"hT_{b}_{c}")
            for r in range(4):
                j = 4 * c + r
                pt = ps_misc.tile([D + 1, P], f32, tag="misc")
                nc.tensor.transpose(pt, hs32[:, j, :], ident)
                nc.scalar.copy(hc[:, r * P : (r + 1) * P], pt)
            hc16 = hpool.tile([D, CH], bf16, tag=f"hT16_{c}", name=f"hT16_{b}_{c}")
            nc.vector.tensor_copy(hc16, hc[0:D, :])
            hTs.append(hc)
            hT16s.append(hc16)

            # qT chunk [65, CH]: rows 0..63 = qT, row 64 = -m (written later).
            qc = qpool.tile([D + 1, CH], p2dt, tag=f"qT{c}", name=f"qT_{b}_{c}")
            pq = ps_misc.tile([D, CH], f32, tag="misc")
            nc.tensor.matmul(
                pq, lhsT=Asb, rhs=hc[0:D, :], start=True, stop=True,
            )
            qc16 = qpool.tile([D, CH], bf16, tag=f"qT16_{c}", name=f"qT16_{b}_{c}")
            nc.scalar.copy(qc16, pq)
            nc.scalar.copy(qc[0:D, :], pq)
            qTs.append(qc)
            qT16s.append(qc16)

        # ---- Stats pass: row max per t-tile ----
        for i in range(NT):
            s_end = (i + 1) * P
            nchunks = _ceil_div(s_end, CH)
            mxp = spool.tile([P, 4], f32, tag="mxp")
            lhs_q = qT16s[i // 4][:, (i % 4) * P : (i % 4 + 1) * P]
            for c in range(nchunks):
                w = min(CH, s_end - c * CH)
                ps = ps_sc.tile([P, CH], f32, tag="sc")
                diag = c == nchunks - 1
                nc.tensor.matmul(
                    ps[:, 0:w], lhsT=lhs_q, rhs=hT16s[c][:, 0:w],
                    start=True, stop=not diag, skip_group_check=True,
                )
                if diag:
                    nc.tensor.matmul(
                        ps[:, w - P : w], lhsT=identb, rhs=umask,
                        start=False, stop=True, skip_group_check=True,
                    )
                nc.vector.reduce_max(
                    mxp[:, c : c + 1], ps[:, 0:w], axis=mybir.AxisListType.X
                )
            negm = spool.tile([P, 1], p2dt, tag="negm")
            if nchunks > 1:
                nc.vector.reduce_max(
                    negm, mxp[:, 0:nchunks], axis=mybir.AxisListType.X, negate=True
                )
            else:
                nc.vector.tensor_scalar_mul(negm, mxp[:, 0:1], -1.0)
            # Partition-column -> free-row reshape via a tiny SBUF->SBUF DMA
            # (keeping this off TensorE avoids transpose-mode matmuls).
            nc.sync.dma_start(
                out=qTs[i // 4][D : D + 1, (i % 4) * P : (i % 4 + 1) * P],
                in_=negm,
            )

        all_hTs.append(hTs)
        all_qTs.append(qTs)

    # ---- Pass 2 + AV for both batches: one long matmul-dense stretch ----
    # outT[c] accumulates [d'+1, t] = sum_j [h_j | 1]^T @ pT(c,j); the
    # ones column of hs16 makes row 64 the softmax denominator l.
    for b in range(B):
        hs16 = hs16s[b]
        hTs = all_hTs[b]
        qTs = all_qTs[b]
        for grp in range(NCH // 2):
            cs = [2 * grp, 2 * grp + 1]
            oTs = {
                c: ps_oT.tile(
                    [D + 1, CH], f32, tag=f"oT{c % 2}", name=f"oT_g{grp}_c{c}"
                )
                for c in cs
            }
            # Software-pipeline: emit each tile's AV two tiles late so the PE
            # (in-order) has independent score matmuls to chew on while ACT
            # computes the exp for this tile.
            av_queue = []

            def finalize_chunk(cq):
                # Runs as soon as chunk cq's accumulation stops, overlapping
                # the remaining AV/score matmuls of the group.
                oc = opool.tile([D + 1, CH], f32, tag="oc", name=f"oc_{b}_{cq}")
                nc.vector.tensor_copy(oc, oTs[cq])
                for r in range(4):
                    ii = 4 * cq + r
                    ot = ps_misc.tile([P, D + 1], f32, tag="misc")
                    nc.tensor.transpose(
                        ot, oc[:, r * P : (r + 1) * P], ident[0 : D + 1, 0 : D + 1]
                    )
                    rl = spool.tile([P, 1], f32, tag="rl")
                    nc.vector.reciprocal(rl, ot[:, D : D + 1])
                    osb = opool.tile([P, D], f32, tag="osb")
                    nc.vector.tensor_scalar_mul(osb, ot[:, 0:D], rl)
                    nc.sync.dma_start(
                        out=out_ext[b, ii * P : (ii + 1) * P, :], in_=osb
                    )

            def flush_av(limit):
                while len(av_queue) > limit:
                    cq, jq, pTq = av_queue.pop(0)
                    nc.tensor.matmul(
                        oTs[cq], lhsT=hs16[:, jq, :], rhs=pTq,
                        start=(jq == 0), stop=(jq == 4 * cq + 3),
                        skip_group_check=True,
                    )
                    if jq == 4 * cq + 3:
                        finalize_chunk(cq)

            for j in range(4 * cs[-1] + 4):
                for c in cs:
                    if j > 4 * c + 3:
                        continue
                    p2 = ps_sc.tile([P, CH], f32, tag="sc")
                    nc.tensor.matmul(
                        p2,
                        lhsT=hTs[j // 4][:, (j % 4) * P : (j % 4 + 1) * P],
                        rhs=qTs[c][:, :],
                        start=True, stop=True, skip_group_check=True,
                    )
                    pT = ppool.tile([P, CH], bf16, tag="pT")
                    nc.scalar.activation(pT, p2, mybir.ActivationFunctionType.Exp)
                    if j >= 4 * c:
                        # zero the acausal part (s_global > t_global); the AV
                        # pipeline delay gives gpsimd slack to do this.
                        nc.gpsimd.affine_select(
                            out=pT, in_=pT, compare_op=mybir.AluOpType.is_ge,
                            fill=0.0, base=-P * (j - 4 * c),
                            channel_multiplier=-1, pattern=[[1, CH]],
                        )
                    av_queue.append((c, j, pT))
                    flush_av(2)
            flush_av(0)


_cache = {}


def _get_nc():
    if "nc" not in _cache:
        nc = bacc.Bacc(
            "TRN2", target_bir_lowering=False, debug=False, num_devices=H
        )
        h_ext = nc.dram_tensor("h", [B, T, D], f32, kind="ExternalInput").ap()
        A_ext = nc.dram_tensor("A", [D, D], f32, kind="ExternalInput").ap()
        out_ext = nc.dram_tensor("out", [B, T, D], f32, kind="ExternalOutput").ap()
        with tile.TileContext(nc) as tc:
            with ExitStack() as ctx:
                _build(ctx, tc, h_ext, A_ext, out_ext)
        nc.compile()
        _cache["nc"] = nc
    return _cache["nc"]


def run(h, A, **kw):
    """Run on hardware; returns (full output [B,T,H*D], BassKernelResults)."""
    nc = _get_nc()
    h = np.ascontiguousarray(h, dtype=np.float32)
    A = np.ascontiguousarray(A, dtype=np.float32)
    in_maps = [{"h": h, "A": np.ascontiguousarray(A[i])} for i in range(H)]
    res = run_bass_kernel_spmd(nc, in_maps, core_ids=list(range(H)), **kw)
    out = np.concatenate([res.results[i]["out"] for i in range(H)], axis=-1)
    return out, res


def kernel(h, A):
    out, _ = run(h, A)
    return out
